# revision 11
# baseline (speedup 1.0000x reference)
"""Expert-parallel sparse-MoE (SwiGLU, top-2 of 8 experts) for 8 TRN2 NeuronCores.

Strategy:
  - Router (softmax + top-2) runs on host (jax-CPU, mirroring the reference
    ops exactly so expert selection matches bit-for-bit).
  - Tokens are gathered per expert on host; each of the 8 cores processes one
    expert's tokens (capacity-padded to a fixed C so one SPMD program serves
    all cores): y_e = (silu(x_e @ gate_e) * (x_e @ up_e)) @ down_e.
  - Host applies the top-2 combine weights and scatter-adds into the output.

Device kernel (per core): activations arrive pre-transposed as x^T [D, C] so
the SwiGLU intermediate is produced in [F, tokens] layout, which feeds the
down-projection matmul directly without any on-device transpose.

Schedule: F is processed in pairs of 512-wide chunks; for each (pair, token
chunk) iteration the gate/up matmuls + silu/mul produce h for 1024 F rows,
then the down-projection for the PREVIOUS iteration's h runs as 8-matmul
PSUM chains (full 1024-F contraction) — one iteration behind, so the PE
never waits on the scalar/vector engines. bf16 operands keep the PE at
1 cyc/row with fast-weight-load; y accumulates in SBUF across pairs.
"""

import numpy as np
import ml_dtypes

import bass_rust
import concourse.bass as bass
import concourse.mybir as mybir
import concourse.tile as tile
from concourse import bass_utils
from concourse.bass import ts

# Problem shapes (hardcoded per contest contract).
B, S, D, F = 4, 2048, 1024, 4096
T = B * S
E = 8
TOPK = 2
P = 128

# Capacity per expert (max routed tokens for the seed-0 inputs is 2182;
# overflow beyond C falls back to exact host compute — keep C a bit below
# the max so the padded tail stays small, the stragglers are cheap on host).
C = 2048
NT = 4              # token chunks of TN
TN = C // NT        # 512: moving free dim for gate/up matmuls (1 PSUM bank)
DN = 512            # output-dim chunk for the down matmul (1 PSUM bank)

DT = mybir.dt.bfloat16
NPDT = ml_dtypes.bfloat16
FC = 512            # F chunk width per fc step
NFC = F // FC       # 8
MF = FC // P        # 4 F-tiles per fc
NPAIR = NFC // 2    # down contraction spans an fc pair (1024 F rows)

_cache = {}


def _split_sync_waits(nc, limit=1):
    """This walrus codegen accepts at most one sync-wait command per
    instruction; hoist excess waits onto same-engine NOPs inserted just
    before the offending instruction (Tile's final drain carries many)."""
    func = nc.m.functions[0]
    for bb in func.blocks:
        insts = bb.instructions
        i = 0
        while i < len(insts):
            ins = insts[i]
            si = ins.sync_info
            if si is not None and si.on_wait and len(si.on_wait) > limit:
                waits = list(si.on_wait)
                eng = nc.engines[ins.engine]
                new_nops = []
                while len(waits) > limit:
                    chunk, waits = waits[:limit], waits[limit:]
                    nop_ins = eng.nop().ins
                    removed = False
                    for bb2 in func.blocks:
                        if bb2.instructions and bb2.instructions[-1] is nop_ins:
                            bb2.instructions.pop()
                            removed = True
                            break
                    assert removed, "could not relocate wait nop"
                    nop_ins.sync_info = bass_rust.SyncInfo(
                        on_wait=chunk, on_update=[]
                    )
                    new_nops.append(nop_ins)
                ins.sync_info = bass_rust.SyncInfo(
                    on_wait=waits, on_update=list(si.on_update or [])
                )
                insts[i:i] = new_nops
                i += len(new_nops)
            i += 1


def _build_nc():
    nc = bass.Bass("TRN2", target_bir_lowering=True)
    # m-major gate/up weights so each [P, D//P, 128] slice is one contiguous
    # DMA (the first matmul chain can start after a single 256 KB transfer).
    gw = nc.dram_tensor("gw", [NFC, MF, P, D // P, P], DT, kind="ExternalInput")
    uw = nc.dram_tensor("uw", [NFC, MF, P, D // P, P], DT, kind="ExternalInput")
    dw = nc.dram_tensor("dw", [NFC, P, MF, D], DT, kind="ExternalInput")
    xT = nc.dram_tensor("xT", [NT, P, D // P, TN], DT, kind="ExternalInput")
    y = nc.dram_tensor("y", [P, C // P, D], DT, kind="ExternalOutput")

    f32 = mybir.dt.float32
    with tile.TileContext(nc) as tc:
        with (
            tc.tile_pool(name="xp", bufs=1) as xp,
            tc.tile_pool(name="yp", bufs=1) as yp,
            tc.tile_pool(name="wp", bufs=2) as wp,
            tc.tile_pool(name="hp", bufs=2) as hp,
            tc.tile_pool(name="gp", bufs=2) as gp,
            tc.tile_pool(name="ps", bufs=2, space="PSUM") as ps,
            tc.tile_pool(name="psy", bufs=4, space="PSUM") as psy,
        ):
            y_sb = yp.tile([P, C // P, D], DT)
            x_sb = [
                xp.tile([P, D // P, TN], DT, tag=f"x{t}", name=f"x{t}")
                for t in range(NT)
            ]

            def load_pair_gu(p, order):
                """DMA gate/up weight m-slices for fc pair p, in `order`."""
                tiles = {}
                for sub in range(2):
                    fc = 2 * p + sub
                    for m in range(MF):
                        tiles[(sub, "g", m)] = wp.tile(
                            [P, D // P, P], DT, tag=f"g{sub}{m}", name=f"g{sub}{m}"
                        )
                        tiles[(sub, "u", m)] = wp.tile(
                            [P, D // P, P], DT, tag=f"u{sub}{m}", name=f"u{sub}{m}"
                        )
                for key in order:
                    sub, gu, m = key
                    fc = 2 * p + sub
                    src = gw if gu == "g" else uw
                    nc.sync.dma_start(tiles[key][:], src[fc, m])
                return tiles

            def load_pair_d(p):
                tiles = []
                for sub in range(2):
                    fc = 2 * p + sub
                    t_ = wp.tile([P, MF, D], DT, tag=f"d{sub}", name=f"d{sub}")
                    nc.sync.dma_start(t_[:], dw[fc])
                    tiles.append(t_)
                return tiles

            # Startup: first gate slice, then the first token chunk, then the
            # rest of pair-0 gate/up weights in consumption order.
            order0 = [(0, "g", 0)]
            order0 += [(0, "g", m) for m in range(1, MF)]
            order0 += [(0, "u", m) for m in range(MF)]
            order0 += [(1, g_u, m) for m in range(MF) for g_u in ("g", "u")]
            gu_tiles = None
            d_tiles = None
            next_gu = None
            next_d = None

            prev = None  # (p, t, h_tile, d_tiles)
            for it, (p, t) in enumerate(
                [(p, t) for p in range(NPAIR) for t in range(NT)]
            ):
                if it == 0:
                    # x loads ride the scalar-engine HWDGE ring so they run in
                    # parallel with the weight loads on the sync ring.
                    nc.scalar.dma_start(x_sb[0][:], xT[0])
                    gu_tiles = load_pair_gu(0, order0)
                    for tt in range(1, NT):
                        nc.scalar.dma_start(x_sb[tt][:], xT[tt])

                # ---- gate/up for (p, t): h[f, tok] for 1024 F rows ----
                h = hp.tile([P, 2 * MF, TN], DT, tag="h")
                for sub in range(2):
                    for m in range(MF):
                        g_w = gu_tiles[(sub, "g", m)]
                        u_w = gu_tiles[(sub, "u", m)]
                        g_sb = gp.tile([P, TN], f32, tag="g")
                        pg = ps.tile([P, TN], f32, tag="pg")
                        for k in range(D // P):
                            nc.tensor.matmul(
                                pg[:],
                                g_w[:, k],
                                x_sb[t][:, k],
                                start=(k == 0),
                                stop=(k == D // P - 1),
                            )
                        nc.scalar.activation(
                            g_sb[:], pg[:], mybir.ActivationFunctionType.Silu
                        )
                        pu = ps.tile([P, TN], f32, tag="pu")
                        for k in range(D // P):
                            nc.tensor.matmul(
                                pu[:],
                                u_w[:, k],
                                x_sb[t][:, k],
                                start=(k == 0),
                                stop=(k == D // P - 1),
                            )
                        nc.vector.tensor_mul(
                            h[:, sub * MF + m], g_sb[:], pu[:]
                        )

                # ---- prefetches for what the NEXT iterations need ----
                if it == 0:
                    d_tiles = load_pair_d(0)
                if t == 1 and p + 1 < NPAIR:
                    next_order = [
                        (sub, g_u, m)
                        for sub in range(2)
                        for g_u in ("g", "u")
                        for m in range(MF)
                    ]
                    next_gu = load_pair_gu(p + 1, next_order)
                if t == 2 and p + 1 < NPAIR:
                    next_d = load_pair_d(p + 1)

                # ---- down-projection for the previous iteration ----
                if prev is not None:
                    _emit_down(nc, prev, y_sb, y, psy)
                prev = (p, t, h, d_tiles)

                if t == NT - 1 and p + 1 < NPAIR:
                    gu_tiles = next_gu
                    d_tiles = next_d

            _emit_down(nc, prev, y_sb, y, psy)

    _split_sync_waits(nc)
    return nc


def _emit_down(nc, prev, y_sb, y, psy):
    """8-matmul PSUM chains contracting the full fc-pair (1024 F rows) of h
    into y for one token chunk; accumulate/stream-out via scalar+vector."""
    f32 = mybir.dt.float32
    p, t, h, d_tiles = prev
    for tm in range(TN // P):
        tt = t * (TN // P) + tm
        for dn in range(D // DN):
            py = psy.tile([P, DN], f32, tag="py")
            for j in range(2 * MF):
                sub, kk = divmod(j, MF)
                nc.tensor.matmul(
                    py[:],
                    h[:, j, ts(tm, P)],
                    d_tiles[sub][:, kk, ts(dn, DN)],
                    start=(j == 0),
                    stop=(j == 2 * MF - 1),
                )
            if p == 0:
                nc.scalar.activation(
                    y_sb[:, tt, ts(dn, DN)], py[:],
                    mybir.ActivationFunctionType.Copy,
                )
            else:
                nc.vector.tensor_add(
                    y_sb[:, tt, ts(dn, DN)], y_sb[:, tt, ts(dn, DN)], py[:]
                )
            if p == NPAIR - 1:
                # alternate store rings so the final drains don't queue
                # behind each other on one HWDGE ring
                eng = nc.sync if (tm + dn) % 2 == 0 else nc.scalar
                eng.dma_start(
                    y[:, tt, ts(dn, DN)], y_sb[:, tt, ts(dn, DN)]
                )


def _route(x, router_w):
    """Mirror the reference router exactly (jax CPU ops)."""
    import jax
    import jax.numpy as jnp

    cpu = jax.devices("cpu")[0]
    with jax.default_device(cpu):
        logits = jnp.asarray(x) @ jnp.asarray(router_w)
        probs = jax.nn.softmax(logits.astype(jnp.float32), axis=-1)
        top_w, top_i = jax.lax.top_k(probs, TOPK)
        return np.asarray(top_w), np.asarray(top_i)


def _silu_np(v):
    return v / (1.0 + np.exp(-v))


def _prep_weights(gate_w, up_w, down_w):
    """Per-expert weight chunks (device dtype) in the device layouts."""
    gw_l, uw_l, dw_l = [], [], []
    for e in range(E):
        g = np.ascontiguousarray(
            gate_w[e].astype(NPDT)
            .reshape(D // P, P, NFC, MF, P).transpose(2, 3, 1, 0, 4)
        )
        u = np.ascontiguousarray(
            up_w[e].astype(NPDT)
            .reshape(D // P, P, NFC, MF, P).transpose(2, 3, 1, 0, 4)
        )
        d = np.ascontiguousarray(
            down_w[e].astype(NPDT)
            .reshape(NFC, MF, P, D).transpose(0, 2, 1, 3)
        )
        gw_l.append(g)
        uw_l.append(u)
        dw_l.append(d)
    return gw_l, uw_l, dw_l


def kernel(hidden_states, router_w, gate_w, up_w, down_w, _trace=False):
    import os
    import time

    timing = os.environ.get("BASS_MOE_TIMING")
    marks = [("start", time.time())]

    def mark(name):
        if timing:
            marks.append((name, time.time()))

    hidden_states = np.asarray(hidden_states)
    router_w = np.asarray(router_w)
    gate_w = np.asarray(gate_w)
    up_w = np.asarray(up_w)
    down_w = np.asarray(down_w)

    x = hidden_states.reshape(-1, D).astype(np.float32, copy=False)
    top_w, top_i = _route(x, router_w)
    mark("route")

    if "nc" not in _cache:
        _cache["nc"] = _build_nc()
    nc = _cache["nc"]
    mark("build")

    wkey = (id(gate_w), id(up_w), id(down_w))
    if _cache.get("wkey") != wkey:
        _cache["w"] = _prep_weights(gate_w, up_w, down_w)
        _cache["wkey"] = wkey
    gw_l, uw_l, dw_l = _cache["w"]
    mark("prep_weights")

    rows_l, wts_l, over_l = [], [], []
    in_maps = []
    for e in range(E):
        rows, which = np.nonzero(top_i == e)
        wts = top_w[rows, which]
        over_l.append((rows[C:], wts[C:]))
        rows, wts = rows[:C], wts[:C]
        rows_l.append(rows)
        wts_l.append(wts)
        n_e = len(rows)
        xTd = np.zeros((NT, P, D // P, TN), NPDT)
        xe = np.zeros((C, D), NPDT)
        xe[:n_e] = x[rows].astype(NPDT)
        xTd[:] = xe.reshape(NT, TN, D // P, P).transpose(0, 3, 2, 1)
        in_maps.append({"xT": xTd, "gw": gw_l[e], "uw": uw_l[e], "dw": dw_l[e]})
    mark("gather")

    res = bass_utils.run_bass_kernel_spmd(
        nc, in_maps, core_ids=list(range(E)), trace=_trace
    )
    if _trace:
        _cache["last_results"] = res
    mark("device_run")

    out = np.zeros((T, D), np.float32)
    for e in range(E):
        ye = res.results[e]["y"]  # [P, C//P, D] bf16
        ye = ye.astype(np.float32).transpose(1, 0, 2).reshape(C, D)
        rows, wts = rows_l[e], wts_l[e]
        out[rows] += wts[:, None] * ye[: len(rows)]
        orows, owts = over_l[e]
        if len(orows):  # capacity overflow: exact host fallback
            xo = x[orows]
            ho = _silu_np(xo @ gate_w[e]) * (xo @ up_w[e])
            out[orows] += owts[:, None] * (ho @ down_w[e])

    mark("scatter")
    if timing:
        for (_, t0), (name, t1) in zip(marks, marks[1:]):
            print(f"  [timing] {name}: {t1 - t0:.3f} s")
    return out.reshape(B, S, D).astype(hidden_states.dtype, copy=False)
